# revision 13
# baseline (speedup 1.0000x reference)
"""Multi-head attention ('general' similarity, softmax, out-proj) on 8 trn2
NeuronCores via Bass/Tile.

Sharding (head-split): core c handles batch b=c//2 and head group hp=c%2
(heads 4hp..4hp+4 = 2 pairs), over the FULL query range.  Each core projects
K/V only for its 4 heads, computes its heads' attention for all 2048 queries,
and emits a PARTIAL (bf16) output projection; the host sums the two partials
per batch in f32.

v2 pipeline (vs the 224.9us baseline):
  * scores emitted interleaved (a0,b0,a1,b1) into FOUR rotating [128,512]
    PSUM chunks so the two row-packed head matmuls stream CONCURRENTLY on
    disjoint PE row-groups and the exp engines never gate the PE stream.
  * exp split per 512-col chunk with a FIXED (head,qchunk)->engine map
    (ACT: a0,b1 table-exp bf16; DVE: a1,b0 bit-trick int16) so the bit-trick
    bias stays common-mode per softmax row.
  * PV accumulates per (head, qchunk) into four [65,512] PSUM banks (ones
    column => rowsums free); part1/part2 run per 512-chunk for fine overlap.
  * all inputs host-prepacked partition-major so every DMA is contiguous
    2-16KB lines; loads ordered by first use; output stored bf16.
"""
import sys
import types

import numpy as np
import ml_dtypes

# ---------------------------------------------------------------- axon shim --
def _ensure_axon_hooks():
    if 'antenv.axon_hooks' in sys.modules:
        return
    try:
        from trn_agent_boot.trn_boot import _ntff_profile_via_ctypes
        hook = _ntff_profile_via_ctypes('/opt/axon/libaxon_pjrt.so')
    except Exception:
        hook = None
    mod = types.ModuleType('antenv.axon_hooks')
    mod.get_axon_ntff_profile_hook = lambda: hook
    mod.set_axon_ntff_profile_hook = lambda h: None
    sys.modules['antenv.axon_hooks'] = mod


_ensure_axon_hooks()

import concourse.bass as bass
import concourse.mybir as mybir
import concourse.tile as tile
from concourse.bass_utils import run_bass_kernel_spmd

BF16 = mybir.dt.bfloat16
F32 = mybir.dt.float32
I16 = mybir.dt.int16

# one-instruction exp on DVE: bf16 bits of exp(s) ~= int16(round(s*128/ln2
# + (127*128 - 5.5))).  Rel err ~N(+1%, 1.8%); the +1% common-mode bias
# cancels in softmax normalization (map is fixed per (head, qchunk) so every
# softmax row is produced by a single engine).
_EXPC1 = 128.0 / 0.6931471805599453
_EXPC2 = 16256.0 - 5.5

P = 128
D = 512          # model dim (= D_K = D_V = D_OUT)
H = 8            # total heads (host-side fold uses this)
SQF = 2048       # full query range per core
SQ = 1024        # query rows per pipeline group (half range)
SK = 2048        # key rows (full sequence)
HL = 4           # local heads per core
DH = 64
DL = HL * DH     # local feature width (256)
NJ = HL // 2     # local head pairs (2)
NG = 4           # pipeline groups: (pair, qhalf)
NKB = SK // P    # 16 key blocks
ND = D // P      # 4 feature blocks
QC = 512         # column chunk (one PSUM bank)
EXPF = mybir.ActivationFunctionType.Exp
LOGF = mybir.ActivationFunctionType.Ln
MULT = mybir.AluOpType.mult
ADD = mybir.AluOpType.add

LAG = 4          # scores -> pv lag in steps
SS = NG * NKB    # 64 pipeline steps
PTB = 2 * (LAG + 2)  # pt tile buffers per engine tag


# ------------------------------------------------------- walrus workaround --
# This container's walrus accepts only ONE embedded sync-wait per hw
# instruction. Move all but the last wait of any instruction onto single-wait
# NoOps inserted just before it in the same engine stream.
_SPLIT_CTR = [0]


def _split_multi_waits(nc, max_waits=1):
    def mk_nop(engine, wait):
        _SPLIT_CTR[0] += 1
        nop = mybir.InstNoOp(name=f"antsplitw-{_SPLIT_CTR[0]}", ins=[], outs=[])
        nop.engine = engine
        nop.sync_info = mybir.SyncInfo(on_wait=[wait], on_update=[])
        return nop

    for fn in nc.m.functions:
        for bb in fn.blocks:
            out = []
            changed = False
            for inst in bb.instructions:
                si = inst.sync_info
                waits = list(si.on_wait) if si is not None and si.on_wait else []
                if len(waits) > max_waits:
                    for w in waits[:-max_waits]:
                        out.append(mk_nop(inst.engine, w))
                    si.on_wait = waits[-max_waits:]
                    changed = True
                out.append(inst)
            if changed:
                bb.instructions = out


# ------------------------------------------------------------ device kernel --
def _build_nc():
    nc = bass.Bass("TRN2", target_bir_lowering=False, debug=False)

    # host-prepacked, partition-major layouts (contiguous DMA lines):
    #   qt/kt: [p, half, qchunk, k, 512]   vt: [p, quarter, k, 512]
    #   wq/wkg/wv: [p, k, 256]   wm: [p, j, 512]
    qt_d = nc.declare_dram_parameter("qt", [P, 2, 2, ND, QC], BF16, isOutput=False)
    kt_d = nc.declare_dram_parameter("kt", [P, 2, 2, ND, QC], BF16, isOutput=False)
    vt_d = nc.declare_dram_parameter("vt", [P, 4, ND, QC], BF16, isOutput=False)
    wq_d = nc.declare_dram_parameter("wq", [P, ND, DL], BF16, isOutput=False)
    # wkg = W_K @ blockdiag(W_gen_S) folded on the host, local columns only
    wkg_d = nc.declare_dram_parameter("wkg", [P, ND, DL], BF16, isOutput=False)
    wv_d = nc.declare_dram_parameter("wv", [P, ND, DL], BF16, isOutput=False)
    wm_d = nc.declare_dram_parameter("wm", [P, NJ, D], BF16, isOutput=False)
    out_d = nc.declare_dram_parameter("out", [SQF, D], BF16, isOutput=True)

    with tile.TileContext(nc) as tc:
        with tc.tile_pool(name="cst", bufs=1) as cst, \
             tc.tile_pool(name="ptp", bufs=1) as ptp, \
             tc.tile_pool(name="dvp", bufs=2) as dvp, \
             tc.tile_pool(name="ps", bufs=1, space="PSUM") as ps:

            # ---- SBUF persistent tiles
            qt = cst.tile([P, 2, 2, ND, QC], BF16, tag="qt")
            kt = cst.tile([P, 2, 2, ND, QC], BF16, tag="kt")
            vt = cst.tile([P, 4, ND, QC], BF16, tag="vt")
            wq = cst.tile([P, ND, DL], BF16, tag="wq")
            wkg = cst.tile([P, ND, DL], BF16, tag="wkg")
            wv = cst.tile([P, ND, DL], BF16, tag="wv")
            wm = cst.tile([P, NJ, D], BF16, tag="wm")

            # loads ordered by first use / landing-time schedule
            nc.sync.dma_start(wq[:], wq_d[:])
            nc.sync.dma_start(qt[:, 0, 0], qt_d[:, 0, 0])
            nc.sync.dma_start(wkg[:], wkg_d[:])
            nc.sync.dma_start(kt[:, 0, 0], kt_d[:, 0, 0])
            nc.sync.dma_start(wv[:], wv_d[:])
            nc.sync.dma_start(vt[:, 0], vt_d[:, 0])
            nc.sync.dma_start(kt[:, 0, 1], kt_d[:, 0, 1])
            nc.sync.dma_start(qt[:, 0, 1], qt_d[:, 0, 1])
            nc.sync.dma_start(kt[:, 1, 0], kt_d[:, 1, 0])
            nc.sync.dma_start(kt[:, 1, 1], kt_d[:, 1, 1])
            nc.sync.dma_start(vt[:, 1], vt_d[:, 1])
            nc.sync.dma_start(qt[:, 1, 0], qt_d[:, 1, 0])
            nc.sync.dma_start(qt[:, 1, 1], qt_d[:, 1, 1])
            nc.sync.dma_start(vt[:, 2], vt_d[:, 2])
            nc.sync.dma_start(vt[:, 3], vt_d[:, 3])
            nc.sync.dma_start(wm[:], wm_d[:])

            # selector for the rowsum-reciprocal partition broadcast with a
            # K=33 contraction: row 0 -> output partitions 0:64 (head a),
            # row 32 -> partitions 64:128 (head b)
            sel = cst.tile([33, P], BF16, tag="sel")
            nc.gpsimd.memset(sel[:], 0.0)
            nc.gpsimd.memset(sel[0:1, 0:DH], 1.0)
            nc.gpsimd.memset(sel[32:33, DH:P], 1.0)

            warm = cst.tile([P, QC], BF16, tag="warm")
            nc.vector.memset(warm[:], 0.0)

            # projections land here
            qlt = [cst.tile([P, SQF], BF16, tag=f"qlt{j}", name=f"qlt{j}")
                   for j in range(NJ)]
            khwt = [cst.tile([P, SK], BF16, tag=f"khwt{j}", name=f"khwt{j}")
                    for j in range(NJ)]
            vaug = [cst.tile([P, HL, DH + 1], BF16, tag=f"vaug{i}",
                             name=f"vaug{i}") for i in range(NKB)]
            headt = [cst.tile([P, SQ], BF16, tag=f"headt{g}", name=f"headt{g}")
                     for g in range(NG)]

            def sc_tile(name):
                return ps.tile([P, QC], F32, tag="sc", bufs=4, name=name)

            def warm_mms(n):
                wps = sc_tile("wps")
                for _ in range(n):
                    nc.tensor.matmul(wps[:], warm[:, 0:P], warm[:],
                                     start=True, stop=True)

            # ---- projections (quarter-granular chunks) ----
            def qproj(j, qtr):
                # qlt[j] cols [512*qtr : 512*qtr+512]
                h, c = divmod(qtr, 2)
                pp = sc_tile("ppq")
                for k in range(ND):
                    nc.tensor.matmul(pp[:], wq[:, k, j * P:(j + 1) * P],
                                     qt[:, h, c, k, :],
                                     start=(k == 0), stop=(k == ND - 1))
                nc.scalar.copy(out=qlt[j][:, qtr * QC:(qtr + 1) * QC],
                               in_=pp[:])

            def kproj(j, qtr):
                h, c = divmod(qtr, 2)
                pp = sc_tile("ppk")
                for k in range(ND):
                    nc.tensor.matmul(pp[:], wkg[:, k, j * P:(j + 1) * P],
                                     kt[:, h, c, k, :],
                                     start=(k == 0), stop=(k == ND - 1))
                nc.scalar.copy(out=khwt[j][:, qtr * QC:(qtr + 1) * QC],
                               in_=pp[:])

            def vproj(i):
                # V_l rows [128i, 128i+128), local heads + the ones column
                pp = sc_tile("ppv")
                for k in range(ND):
                    nc.tensor.matmul(pp[:, 0:DL],
                                     vt[:, i // 4, k, (i % 4) * P:(i % 4 + 1) * P],
                                     wv[:, k, :],
                                     start=(k == 0), stop=(k == ND - 1))
                nc.vector.tensor_copy(
                    out=vaug[i][:, :, 0:DH],
                    in_=pp[:, 0:DL].rearrange("p (h v) -> p h v", v=DH))
                nc.gpsimd.memset(vaug[i][:, :, DH:DH + 1], 1.0)

            def outproj(qb):
                # partial out-proj for query block qb (128 q rows)
                qh = qb // 8
                po = sc_tile("po")
                for pr in range(NJ):
                    g = pr * 2 + qh
                    nc.tensor.matmul(po[:],
                                     headt[g][:, (qb % 8) * P:(qb % 8 + 1) * P],
                                     wm[:, pr, :], start=(pr == 0),
                                     stop=(pr == NJ - 1))
                ot = dvp.tile([P, D], BF16, tag="ot", bufs=4, name="ot")
                if qb % 2 == 0:
                    nc.scalar.copy(out=ot[:], in_=po[:])
                else:
                    nc.vector.tensor_copy(out=ot[:], in_=po[:])
                nc.sync.dma_start(out_d[qb * P:(qb + 1) * P, :], ot[:])

            # ---- attention pipeline over steps pos = g*16 + t ----
            # chunks per step: (head, qc) with fixed exp engine map
            #   (0,0)->ACT  (0,1)->DVE  (1,0)->DVE  (1,1)->ACT
            pts = {}        # (pos, head, qc) -> pt tile
            pv_tiles = {}   # (g, head, qc) -> accumulator
            recrs = {}      # (g, qc) -> recr tile

            def scores_step(pos):
                g, t = divmod(pos, NKB)
                pr, qh = divmod(g, 2)
                q0 = qh * SQ
                # emit interleaved a,b per qc so the row-packed pair streams
                # concurrently (disjoint PE row groups)
                for qc in range(2):
                    s = q0 + qc * QC
                    sca = sc_tile("sca")
                    scb = sc_tile("scb")
                    nc.tensor.matmul(
                        sca[:], khwt[pr][0:DH, t * P:(t + 1) * P],
                        qlt[pr][0:DH, s:s + QC], start=True, stop=True)
                    nc.tensor.matmul(
                        scb[:], khwt[pr][DH:P, t * P:(t + 1) * P],
                        qlt[pr][DH:P, s:s + QC], start=True, stop=True,
                        tile_position=(DH, 0))
                    # exp: head a chunk
                    if qc == 0:
                        pta = ptp.tile([P, QC], BF16, tag="pt_act", bufs=PTB,
                                       name="pta")
                        nc.scalar.activation(pta[:], sca[:], EXPF)
                    else:
                        pta = ptp.tile([P, QC], I16, tag="pt_dve", bufs=PTB,
                                       name="pta")
                        nc.vector.tensor_scalar(pta[:], sca[:], _EXPC1,
                                                _EXPC2, MULT, ADD)
                    pts[(pos, 0, qc)] = pta
                    # exp: head b chunk
                    if qc == 0:
                        ptb = ptp.tile([P, QC], I16, tag="pt_dve", bufs=PTB,
                                       name="ptb")
                        nc.vector.tensor_scalar(ptb[:], scb[:], _EXPC1,
                                                _EXPC2, MULT, ADD)
                    else:
                        ptb = ptp.tile([P, QC], BF16, tag="pt_act", bufs=PTB,
                                       name="ptb")
                        nc.scalar.activation(ptb[:], scb[:], EXPF)
                    pts[(pos, 1, qc)] = ptb

            def pv_step(pos):
                g, t = divmod(pos, NKB)
                pr = g // 2
                if t == 0:
                    for head in range(2):
                        for qc in range(2):
                            pv_tiles[(g, head, qc)] = ps.tile(
                                [DH + 1, QC], F32, tag=f"pv{head}{qc}",
                                bufs=1, name=f"pv{head}{qc}")
                st, sp = (t == 0), (t == NKB - 1)
                # per stationary (head) so each vaug head is loaded once
                for head in range(2):
                    for qc in range(2):
                        pt = pts.pop((pos, head, qc))
                        mv = pt[:] if pt.dtype == BF16 else pt[:].bitcast(BF16)
                        nc.tensor.matmul(pv_tiles[(g, head, qc)][:],
                                         vaug[t][:, 2 * pr + head, :],
                                         mv, start=st, stop=sp)

            def part1(g, qc):
                # 1/rowsum = exp(-ln(rowsum)), rowsum rows read straight from
                # PSUM (row 64 of each accumulator)
                pva = pv_tiles[(g, 0, qc)]
                pvb = pv_tiles[(g, 1, qc)]
                lg = dvp.tile([33, QC], F32, tag="lg", bufs=2, name="lg")
                nc.gpsimd.memset(lg[:], 1.0)
                nc.scalar.activation(lg[0:1, :], pva[DH:DH + 1, :], LOGF)
                nc.scalar.activation(lg[32:33, :], pvb[DH:DH + 1, :], LOGF)
                recr = dvp.tile([33, QC], BF16, tag="recr", bufs=2, name="recr")
                nc.scalar.activation(recr[:], lg[:], EXPF, scale=-1.0)
                recrs[(g, qc)] = recr

            def part2(g, qc):
                pva = pv_tiles.pop((g, 0, qc))
                pvb = pv_tiles.pop((g, 1, qc))
                recr = recrs.pop((g, qc))
                rbp = sc_tile("rbp")
                nc.tensor.matmul(rbp[:], sel[:], recr[:], start=True, stop=True)
                rbe = dvp.tile([DH, QC], F32, tag="rbe", bufs=2, name="rbe")
                rbo = dvp.tile([DH, QC], F32, tag="rbo", bufs=2, name="rbo")
                nc.scalar.copy(out=rbe[:], in_=rbp[0:DH, :])
                nc.vector.tensor_copy(out=rbo[:], in_=rbp[DH:P, :])
                s = qc * QC
                nc.vector.tensor_tensor(headt[g][0:DH, s:s + QC],
                                        pva[0:DH, :], rbe[:], MULT)
                nc.vector.tensor_tensor(headt[g][DH:P, s:s + QC],
                                        pvb[0:DH, :], rbo[:], MULT)

            # ---- static schedule ----
            pre = {}
            pre2 = {}

            def at(pos, f):
                pre.setdefault(pos, []).append(f)

            def at2(pos, f):
                pre2.setdefault(pos, []).append(f)

            # early schedule matched to DMA landing order; extras in the
            # DMA-gated window where engines idle anyway
            at(1, lambda: vproj(0))
            at(1, lambda: warm_mms(1))
            at(2, lambda: vproj(1))
            at(2, lambda: kproj(0, 1))
            at(3, lambda: vproj(2))
            at(3, lambda: warm_mms(1))
            at(4, lambda: vproj(3))
            at(4, lambda: kproj(0, 2))
            at(5, lambda: kproj(0, 3))
            at(5, lambda: warm_mms(1))
            at(6, lambda: vproj(4))
            at(6, lambda: qproj(1, 0))
            at(7, lambda: vproj(5))
            at(7, lambda: warm_mms(1))
            at(8, lambda: vproj(6))
            at(9, lambda: vproj(7))
            at(9, lambda: warm_mms(1))
            at(10, lambda: vproj(8))
            at(10, lambda: qproj(1, 1))
            at(11, lambda: vproj(9))
            at(11, lambda: qproj(0, 2))
            at(12, lambda: vproj(10))
            at(12, lambda: qproj(0, 3))
            at(13, lambda: vproj(11))
            at(13, lambda: vproj(12))
            at(14, lambda: vproj(13))
            at(14, lambda: vproj(14))
            at(15, lambda: vproj(15))
            at(21, lambda: kproj(1, 0))
            at(21, lambda: qproj(1, 2))
            at(22, lambda: kproj(1, 1))
            at(22, lambda: qproj(1, 3))
            at(23, lambda: kproj(1, 2))
            at(24, lambda: kproj(1, 3))
            # boundary normalize for groups 0..2: part1 (ACT-only) right
            # after the tapered pv at E+17; part2's sel-matmuls go BEFORE the
            # next positions' scores (pre2) behind warm fillers
            for g in range(NG - 1):
                E = NKB * g
                at(E + 17, lambda g=g: part1(g, 0))
                at(E + 17, lambda g=g: part1(g, 1))
                at(E + 17, lambda: warm_mms(2))
                at2(E + 18, lambda: warm_mms(2))
                at2(E + 18, lambda g=g: part2(g, 0))
                at2(E + 19, lambda: warm_mms(1))
                at2(E + 19, lambda g=g: part2(g, 1))
                at2(E + 20, lambda: warm_mms(1))
            # qhalf-0 output blocks (need part2 of groups 0 and 2)
            for k in range(4):
                at(54 + 2 * k, lambda k=k: outproj(k))
                at(55 + 2 * k, lambda k=k: outproj(4 + k))

            # prologue: warm the PE clock-gate with dummy matmuls while the
            # first DMAs land, then emit what group 0's first steps need
            warm_mms(10)
            qproj(0, 0)
            kproj(0, 0)
            qproj(0, 1)

            # pv lag taper: groups 0..2 land their last two pv steps at pos
            # 16g+17 so part1/part2 can run before the next group needs the
            # psum banks; group 3 tapers early so the tail drain is short
            pv_at = {}
            for p_ in range(SS):
                g, tl = divmod(p_, NKB)
                if g == NG - 1:
                    lag = {10: 3, 11: 3, 12: 2, 13: 2, 14: 1, 15: 1}.get(tl, LAG)
                else:
                    lag = {14: 3, 15: 2}.get(tl, LAG)
                pv_at.setdefault(p_ + lag, []).append(p_)

            for pos in range(SS + 1):
                for f in pre2.get(pos, []):
                    f()
                if pos < SS:
                    scores_step(pos)
                for p_ in pv_at.get(pos, []):
                    pv_step(p_)
                for f in pre.get(pos, []):
                    f()

            # ---- tail: group 3 normalize + qhalf-1 output blocks ----
            gl = NG - 1
            part1(gl, 0)
            warm_mms(4)
            part1(gl, 1)
            part2(gl, 0)
            warm_mms(2)
            for qb in range(8, 12):
                outproj(qb)
            part2(gl, 1)
            for qb in range(12, 16):
                outproj(qb)

    _split_multi_waits(nc)
    return nc


_NC = None


def _get_nc():
    global _NC
    if _NC is None:
        _NC = _build_nc()
    return _NC


def _prep_in_maps(Q, K, V, W_Q, W_K, W_V, W_gen_S, W_multi_head):
    bf = ml_dtypes.bfloat16
    wq_f = np.asarray(W_Q, np.float32)
    wv_f = np.asarray(W_V, np.float32)
    wm_f = np.asarray(W_multi_head, np.float32)
    # fold W_gen_S into W_K: K_hw = K @ W_K @ blockdiag(W_gen_S)
    wk_f = np.asarray(W_K, np.float32)
    wg_f = np.asarray(W_gen_S, np.float32)
    wkg_f = np.einsum('dhe,ef->dhf', wk_f.reshape(D, H, DH), wg_f)
    wkg_f = wkg_f.reshape(D, D)

    Q = np.asarray(Q, np.float32)
    K = np.asarray(K, np.float32)
    V = np.asarray(V, np.float32)

    def pack_qk(X):
        # [2048 s, 512 d] -> XT [512, 2048] -> [p, half, qc, k, 512]
        XT = X.T.reshape(ND, P, 2, 2, QC)
        return np.ascontiguousarray(XT.transpose(1, 2, 3, 0, 4)).astype(bf)

    def pack_v(X):
        # -> [p, quarter, k, 512]
        XT = X.T.reshape(ND, P, 4, QC)
        return np.ascontiguousarray(XT.transpose(1, 2, 0, 3)).astype(bf)

    def pack_w(Wl):
        # [512 d, 256 e] -> [p, k, 256]
        return np.ascontiguousarray(
            Wl.reshape(ND, P, DL).transpose(1, 0, 2)).astype(bf)

    qts = [pack_qk(Q[b]) for b in range(4)]
    kts = [pack_qk(K[b]) for b in range(4)]
    vts = [pack_v(V[b]) for b in range(4)]

    in_maps = []
    for c in range(8):
        b, hp = divmod(c, 2)
        sl = slice(hp * DL, (hp + 1) * DL)
        wm_l = wm_f[sl, :]  # [256, 512]
        in_maps.append({
            "qt": qts[b], "kt": kts[b], "vt": vts[b],
            "wq": pack_w(wq_f[:, sl]),
            "wkg": pack_w(wkg_f[:, sl]),
            "wv": pack_w(wv_f[:, sl]),
            "wm": np.ascontiguousarray(
                wm_l.reshape(NJ, P, D).transpose(1, 0, 2)).astype(bf),
        })
    return in_maps


def _run(in_maps, trace=False):
    nc = _get_nc()
    res = run_bass_kernel_spmd(nc, in_maps, list(range(8)), trace=trace)
    out = np.empty((4, SQF, D), np.float32)
    for b in range(4):
        out[b] = (res.results[2 * b]["out"].astype(np.float32)
                  + res.results[2 * b + 1]["out"].astype(np.float32))
    return out, res


def kernel(Q, K, V, M, W_Q, W_K, W_V, W_gen_S, W_multi_head):
    in_maps = _prep_in_maps(Q, K, V, W_Q, W_K, W_V, W_gen_S, W_multi_head)
    out, _ = _run(in_maps, trace=False)
    return out


def kernel_traced(Q, K, V, M, W_Q, W_K, W_V, W_gen_S, W_multi_head):
    in_maps = _prep_in_maps(Q, K, V, W_Q, W_K, W_V, W_gen_S, W_multi_head)
    return _run(in_maps, trace=True)


# revision 14
# speedup vs baseline: 1.0144x; 1.0144x over previous
"""Multi-head attention ('general' similarity, softmax, out-proj) on 8 trn2
NeuronCores via Bass/Tile.

Sharding (head-split): core c handles batch b=c//2 and head group hp=c%2
(heads 4hp..4hp+4 = 2 pairs), over the FULL query range.  Each core projects
K/V only for its 4 heads, computes its heads' attention for all 2048 queries,
and emits a PARTIAL (bf16) output projection; the host sums the two partials
per batch in f32.

v2 pipeline (vs the 224.9us baseline):
  * scores emitted interleaved (a0,b0,a1,b1) into FOUR rotating [128,512]
    PSUM chunks so the two row-packed head matmuls stream CONCURRENTLY on
    disjoint PE row-groups and the exp engines never gate the PE stream.
  * exp split per 512-col chunk with a FIXED (head,qchunk)->engine map
    (ACT: a0,b1 table-exp bf16; DVE: a1,b0 bit-trick int16) so the bit-trick
    bias stays common-mode per softmax row.
  * PV accumulates per (head, qchunk) into four [65,512] PSUM banks (ones
    column => rowsums free); part1/part2 run per 512-chunk for fine overlap.
  * all inputs host-prepacked partition-major so every DMA is contiguous
    2-16KB lines; loads ordered by first use; output stored bf16.
"""
import sys
import types

import numpy as np
import ml_dtypes

# ---------------------------------------------------------------- axon shim --
def _ensure_axon_hooks():
    if 'antenv.axon_hooks' in sys.modules:
        return
    try:
        from trn_agent_boot.trn_boot import _ntff_profile_via_ctypes
        hook = _ntff_profile_via_ctypes('/opt/axon/libaxon_pjrt.so')
    except Exception:
        hook = None
    mod = types.ModuleType('antenv.axon_hooks')
    mod.get_axon_ntff_profile_hook = lambda: hook
    mod.set_axon_ntff_profile_hook = lambda h: None
    sys.modules['antenv.axon_hooks'] = mod


_ensure_axon_hooks()

import concourse.bass as bass
import concourse.mybir as mybir
import concourse.tile as tile
from concourse.bass_utils import run_bass_kernel_spmd

BF16 = mybir.dt.bfloat16
F32 = mybir.dt.float32
I16 = mybir.dt.int16

# one-instruction exp on DVE: bf16 bits of exp(s) ~= int16(round(s*128/ln2
# + (127*128 - 5.5))).  Rel err ~N(+1%, 1.8%); the +1% common-mode bias
# cancels in softmax normalization (map is fixed per (head, qchunk) so every
# softmax row is produced by a single engine).
_EXPC1 = 128.0 / 0.6931471805599453
_EXPC2 = 16256.0 - 5.5

P = 128
D = 512          # model dim (= D_K = D_V = D_OUT)
H = 8            # total heads (host-side fold uses this)
SQF = 2048       # full query range per core
SQ = 1024        # query rows per pipeline group (half range)
SK = 2048        # key rows (full sequence)
HL = 4           # local heads per core
DH = 64
DL = HL * DH     # local feature width (256)
NJ = HL // 2     # local head pairs (2)
NG = 4           # pipeline groups: (pair, qhalf)
NKB = SK // P    # 16 key blocks
ND = D // P      # 4 feature blocks
QC = 512         # column chunk (one PSUM bank)
EXPF = mybir.ActivationFunctionType.Exp
LOGF = mybir.ActivationFunctionType.Ln
MULT = mybir.AluOpType.mult
ADD = mybir.AluOpType.add

LAG = 4          # scores -> pv lag in steps
SS = NG * NKB    # 64 pipeline steps
PTB = 16         # pt tile buffers per engine tag (covers boundary lag-in of 6)


# ------------------------------------------------------- walrus workaround --
# This container's walrus accepts only ONE embedded sync-wait per hw
# instruction. Move all but the last wait of any instruction onto single-wait
# NoOps inserted just before it in the same engine stream.
_SPLIT_CTR = [0]


def _split_multi_waits(nc, max_waits=1):
    def mk_nop(engine, wait):
        _SPLIT_CTR[0] += 1
        nop = mybir.InstNoOp(name=f"antsplitw-{_SPLIT_CTR[0]}", ins=[], outs=[])
        nop.engine = engine
        nop.sync_info = mybir.SyncInfo(on_wait=[wait], on_update=[])
        return nop

    for fn in nc.m.functions:
        for bb in fn.blocks:
            out = []
            changed = False
            for inst in bb.instructions:
                si = inst.sync_info
                waits = list(si.on_wait) if si is not None and si.on_wait else []
                if len(waits) > max_waits:
                    for w in waits[:-max_waits]:
                        out.append(mk_nop(inst.engine, w))
                    si.on_wait = waits[-max_waits:]
                    changed = True
                out.append(inst)
            if changed:
                bb.instructions = out


# ------------------------------------------------------------ device kernel --
def _build_nc():
    nc = bass.Bass("TRN2", target_bir_lowering=False, debug=False)

    # host-prepacked, partition-major layouts (contiguous DMA lines):
    #   qt/kt: [p, half, qchunk, k, 512]   vt: [p, quarter, k, 512]
    #   wq/wkg/wv: [p, k, 256]   wm: [p, j, 512]
    qt_d = nc.declare_dram_parameter("qt", [P, 2, 2, ND, QC], BF16, isOutput=False)
    kt_d = nc.declare_dram_parameter("kt", [P, 2, 2, ND, QC], BF16, isOutput=False)
    vt_d = nc.declare_dram_parameter("vt", [P, 4, ND, QC], BF16, isOutput=False)
    wq_d = nc.declare_dram_parameter("wq", [P, ND, DL], BF16, isOutput=False)
    # wkg = W_K @ blockdiag(W_gen_S) folded on the host, local columns only
    wkg_d = nc.declare_dram_parameter("wkg", [P, ND, DL], BF16, isOutput=False)
    wv_d = nc.declare_dram_parameter("wv", [P, ND, DL], BF16, isOutput=False)
    wm_d = nc.declare_dram_parameter("wm", [P, NJ, D], BF16, isOutput=False)
    out_d = nc.declare_dram_parameter("out", [SQF, D], BF16, isOutput=True)

    with tile.TileContext(nc) as tc:
        with tc.tile_pool(name="cst", bufs=1) as cst, \
             tc.tile_pool(name="ptp", bufs=1) as ptp, \
             tc.tile_pool(name="dvp", bufs=2) as dvp, \
             tc.tile_pool(name="ps", bufs=1, space="PSUM") as ps:

            # ---- SBUF persistent tiles
            qt = cst.tile([P, 2, 2, ND, QC], BF16, tag="qt")
            kt = cst.tile([P, 2, 2, ND, QC], BF16, tag="kt")
            vt = cst.tile([P, 4, ND, QC], BF16, tag="vt")
            wq = cst.tile([P, ND, DL], BF16, tag="wq")
            wkg = cst.tile([P, ND, DL], BF16, tag="wkg")
            wv = cst.tile([P, ND, DL], BF16, tag="wv")
            wm = cst.tile([P, NJ, D], BF16, tag="wm")

            # loads ordered by first use / landing-time schedule
            nc.sync.dma_start(wq[:], wq_d[:])
            nc.sync.dma_start(qt[:, 0, 0], qt_d[:, 0, 0])
            nc.sync.dma_start(wkg[:], wkg_d[:])
            nc.sync.dma_start(kt[:, 0, 0], kt_d[:, 0, 0])
            nc.sync.dma_start(qt[:, 0, 1], qt_d[:, 0, 1])
            nc.sync.dma_start(wv[:], wv_d[:])
            nc.sync.dma_start(vt[:, 0], vt_d[:, 0])
            nc.sync.dma_start(kt[:, 0, 1], kt_d[:, 0, 1])
            nc.sync.dma_start(kt[:, 1, 0], kt_d[:, 1, 0])
            nc.sync.dma_start(kt[:, 1, 1], kt_d[:, 1, 1])
            nc.sync.dma_start(vt[:, 1], vt_d[:, 1])
            nc.sync.dma_start(qt[:, 1, 0], qt_d[:, 1, 0])
            nc.sync.dma_start(qt[:, 1, 1], qt_d[:, 1, 1])
            nc.sync.dma_start(vt[:, 2], vt_d[:, 2])
            nc.sync.dma_start(vt[:, 3], vt_d[:, 3])
            nc.sync.dma_start(wm[:], wm_d[:])

            # selector for the rowsum-reciprocal partition broadcast with a
            # K=33 contraction: row 0 -> output partitions 0:64 (head a),
            # row 32 -> partitions 64:128 (head b)
            sel = cst.tile([33, P], BF16, tag="sel")
            nc.gpsimd.memset(sel[:], 0.0)
            nc.gpsimd.memset(sel[0:1, 0:DH], 1.0)
            nc.gpsimd.memset(sel[32:33, DH:P], 1.0)

            warm = cst.tile([P, QC], BF16, tag="warm")
            nc.vector.memset(warm[:], 0.0)

            # projections land here
            qlt = [cst.tile([P, SQF], BF16, tag=f"qlt{j}", name=f"qlt{j}")
                   for j in range(NJ)]
            khwt = [cst.tile([P, SK], BF16, tag=f"khwt{j}", name=f"khwt{j}")
                    for j in range(NJ)]
            vaug = [cst.tile([P, HL, DH + 1], BF16, tag=f"vaug{i}",
                             name=f"vaug{i}") for i in range(NKB)]
            headt = [cst.tile([P, SQ], BF16, tag=f"headt{g}", name=f"headt{g}")
                     for g in range(NG)]

            def sc_tile(name):
                return ps.tile([P, QC], F32, tag="sc", bufs=4, name=name)

            def warm_mms(n):
                wps = sc_tile("wps")
                for _ in range(n):
                    nc.tensor.matmul(wps[:], warm[:, 0:P], warm[:],
                                     start=True, stop=True)

            # ---- projections (quarter-granular chunks) ----
            def qproj(j, qtr):
                # qlt[j] cols [512*qtr : 512*qtr+512]
                h, c = divmod(qtr, 2)
                pp = sc_tile("ppq")
                for k in range(ND):
                    nc.tensor.matmul(pp[:], wq[:, k, j * P:(j + 1) * P],
                                     qt[:, h, c, k, :],
                                     start=(k == 0), stop=(k == ND - 1))
                nc.scalar.copy(out=qlt[j][:, qtr * QC:(qtr + 1) * QC],
                               in_=pp[:])

            def kproj(j, qtr):
                h, c = divmod(qtr, 2)
                pp = sc_tile("ppk")
                for k in range(ND):
                    nc.tensor.matmul(pp[:], wkg[:, k, j * P:(j + 1) * P],
                                     kt[:, h, c, k, :],
                                     start=(k == 0), stop=(k == ND - 1))
                nc.scalar.copy(out=khwt[j][:, qtr * QC:(qtr + 1) * QC],
                               in_=pp[:])

            def vproj(i):
                # V_l rows [128i, 128i+128), local heads + the ones column
                pp = sc_tile("ppv")
                for k in range(ND):
                    nc.tensor.matmul(pp[:, 0:DL],
                                     vt[:, i // 4, k, (i % 4) * P:(i % 4 + 1) * P],
                                     wv[:, k, :],
                                     start=(k == 0), stop=(k == ND - 1))
                nc.vector.tensor_copy(
                    out=vaug[i][:, :, 0:DH],
                    in_=pp[:, 0:DL].rearrange("p (h v) -> p h v", v=DH))
                nc.gpsimd.memset(vaug[i][:, :, DH:DH + 1], 1.0)

            def outproj(qb):
                # partial out-proj for query block qb (128 q rows)
                qh = qb // 8
                po = sc_tile("po")
                for pr in range(NJ):
                    g = pr * 2 + qh
                    nc.tensor.matmul(po[:],
                                     headt[g][:, (qb % 8) * P:(qb % 8 + 1) * P],
                                     wm[:, pr, :], start=(pr == 0),
                                     stop=(pr == NJ - 1))
                ot = dvp.tile([P, D], BF16, tag="ot", bufs=4, name="ot")
                if qb % 2 == 0:
                    nc.scalar.copy(out=ot[:], in_=po[:])
                else:
                    nc.vector.tensor_copy(out=ot[:], in_=po[:])
                nc.sync.dma_start(out_d[qb * P:(qb + 1) * P, :], ot[:])

            # ---- attention pipeline over steps pos = g*16 + t ----
            # chunks per step: (head, qc) with fixed exp engine map
            #   (0,0)->ACT  (0,1)->DVE  (1,0)->DVE  (1,1)->ACT
            pts = {}        # (pos, head, qc) -> pt tile
            pv_tiles = {}   # (g, head, qc) -> accumulator
            recrs = {}      # (g, qc) -> recr tile

            def scores_step(pos):
                g, t = divmod(pos, NKB)
                pr, qh = divmod(g, 2)
                q0 = qh * SQ
                # emit interleaved a,b per qc so the row-packed pair streams
                # concurrently (disjoint PE row groups)
                for qc in range(2):
                    s = q0 + qc * QC
                    sca = sc_tile("sca")
                    scb = sc_tile("scb")
                    nc.tensor.matmul(
                        sca[:], khwt[pr][0:DH, t * P:(t + 1) * P],
                        qlt[pr][0:DH, s:s + QC], start=True, stop=True)
                    nc.tensor.matmul(
                        scb[:], khwt[pr][DH:P, t * P:(t + 1) * P],
                        qlt[pr][DH:P, s:s + QC], start=True, stop=True,
                        tile_position=(DH, 0))
                    # exp: head a chunk
                    if qc == 0:
                        pta = ptp.tile([P, QC], BF16, tag="pt_act", bufs=PTB,
                                       name="pta")
                        nc.scalar.activation(pta[:], sca[:], EXPF)
                    else:
                        pta = ptp.tile([P, QC], I16, tag="pt_dve", bufs=PTB,
                                       name="pta")
                        nc.vector.tensor_scalar(pta[:], sca[:], _EXPC1,
                                                _EXPC2, MULT, ADD)
                    pts[(pos, 0, qc)] = pta
                    # exp: head b chunk
                    if qc == 0:
                        ptb = ptp.tile([P, QC], I16, tag="pt_dve", bufs=PTB,
                                       name="ptb")
                        nc.vector.tensor_scalar(ptb[:], scb[:], _EXPC1,
                                                _EXPC2, MULT, ADD)
                    else:
                        ptb = ptp.tile([P, QC], BF16, tag="pt_act", bufs=PTB,
                                       name="ptb")
                        nc.scalar.activation(ptb[:], scb[:], EXPF)
                    pts[(pos, 1, qc)] = ptb

            def pv_step(pos):
                g, t = divmod(pos, NKB)
                pr = g // 2
                if t == 0:
                    for head in range(2):
                        for qc in range(2):
                            pv_tiles[(g, head, qc)] = ps.tile(
                                [DH + 1, QC], F32, tag=f"pv{head}{qc}",
                                bufs=1, name=f"pv{head}{qc}")
                st, sp = (t == 0), (t == NKB - 1)
                # per stationary (head) so each vaug head is loaded once
                for head in range(2):
                    for qc in range(2):
                        pt = pts.pop((pos, head, qc))
                        mv = pt[:] if pt.dtype == BF16 else pt[:].bitcast(BF16)
                        nc.tensor.matmul(pv_tiles[(g, head, qc)][:],
                                         vaug[t][:, 2 * pr + head, :],
                                         mv, start=st, stop=sp)

            def part1(g, qc):
                # 1/rowsum = exp(-ln(rowsum)), rowsum rows read straight from
                # PSUM (row 64 of each accumulator)
                pva = pv_tiles[(g, 0, qc)]
                pvb = pv_tiles[(g, 1, qc)]
                lg = dvp.tile([33, QC], F32, tag="lg", bufs=2, name="lg")
                nc.gpsimd.memset(lg[:], 1.0)
                nc.scalar.activation(lg[0:1, :], pva[DH:DH + 1, :], LOGF)
                nc.scalar.activation(lg[32:33, :], pvb[DH:DH + 1, :], LOGF)
                recr = dvp.tile([33, QC], BF16, tag="recr", bufs=2, name="recr")
                nc.scalar.activation(recr[:], lg[:], EXPF, scale=-1.0)
                recrs[(g, qc)] = recr

            def part2(g, qc):
                pva = pv_tiles.pop((g, 0, qc))
                pvb = pv_tiles.pop((g, 1, qc))
                recr = recrs.pop((g, qc))
                rbp = sc_tile("rbp")
                nc.tensor.matmul(rbp[:], sel[:], recr[:], start=True, stop=True)
                rbe = dvp.tile([DH, QC], F32, tag="rbe", bufs=2, name="rbe")
                rbo = dvp.tile([DH, QC], F32, tag="rbo", bufs=2, name="rbo")
                nc.scalar.copy(out=rbe[:], in_=rbp[0:DH, :])
                nc.vector.tensor_copy(out=rbo[:], in_=rbp[DH:P, :])
                s = qc * QC
                nc.vector.tensor_tensor(headt[g][0:DH, s:s + QC],
                                        pva[0:DH, :], rbe[:], MULT)
                nc.vector.tensor_tensor(headt[g][DH:P, s:s + QC],
                                        pvb[0:DH, :], rbo[:], MULT)

            # ---- static schedule ----
            pre = {}
            pre2 = {}

            def at(pos, f):
                pre.setdefault(pos, []).append(f)

            def at2(pos, f):
                pre2.setdefault(pos, []).append(f)

            # early schedule matched to DMA landing order; extras in the
            # DMA-gated window where engines idle anyway
            at(1, lambda: vproj(0))
            at(1, lambda: warm_mms(1))
            at(2, lambda: vproj(1))
            at(2, lambda: kproj(0, 1))
            at(3, lambda: vproj(2))
            at(3, lambda: warm_mms(1))
            at(4, lambda: vproj(3))
            at(4, lambda: kproj(0, 2))
            at(5, lambda: kproj(0, 3))
            at(5, lambda: warm_mms(1))
            at(6, lambda: vproj(4))
            at(6, lambda: qproj(1, 0))
            at(7, lambda: vproj(5))
            at(7, lambda: warm_mms(1))
            at(8, lambda: vproj(6))
            at(9, lambda: vproj(7))
            at(9, lambda: warm_mms(1))
            at(10, lambda: vproj(8))
            at(10, lambda: qproj(1, 1))
            at(11, lambda: vproj(9))
            at(11, lambda: qproj(0, 2))
            at(12, lambda: vproj(10))
            at(12, lambda: qproj(0, 3))
            at(13, lambda: vproj(11))
            at(13, lambda: vproj(12))
            at(14, lambda: vproj(13))
            at(14, lambda: vproj(14))
            at(15, lambda: vproj(15))
            at(21, lambda: kproj(1, 0))
            at(21, lambda: qproj(1, 2))
            at(22, lambda: kproj(1, 1))
            at(22, lambda: qproj(1, 3))
            at(23, lambda: kproj(1, 2))
            at(24, lambda: kproj(1, 3))
            # boundary normalize for groups 0..2: pv taper lands the last
            # pv steps at E+16/E+17, part1 right after, part2 at E+19/E+20;
            # the next group's pv eases in at E+22 (lag 6).  Fillers keep
            # the PE clock-gate warm through the chain.
            for g in range(NG - 1):
                E = NKB * g
                at(E + 17, lambda g=g: part1(g, 0))
                at(E + 17, lambda g=g: part1(g, 1))
                at(E + 18, lambda: warm_mms(3))
                at(E + 19, lambda g=g: part2(g, 0))
                at(E + 19, lambda: warm_mms(2))
                at(E + 20, lambda g=g: part2(g, 1))
                at(E + 20, lambda: warm_mms(2))
                at(E + 21, lambda: warm_mms(3))
            # qhalf-0 output blocks (need part2 of groups 0 and 2)
            for k in range(4):
                at(54 + 2 * k, lambda k=k: outproj(k))
                at(55 + 2 * k, lambda k=k: outproj(4 + k))

            # prologue: warm the PE clock-gate with dummy matmuls while the
            # first DMAs land, then emit what group 0's first steps need
            warm_mms(10)
            qproj(0, 0)
            warm_mms(4)
            kproj(0, 0)
            warm_mms(4)
            qproj(0, 1)

            # pv lag schedule: groups 0..2 taper out over E+16/E+17 so part1
            # can start early; each group after the first eases in at lag 6
            # so the psum banks have time to clear through part2
            pv_at = {}
            for p_ in range(SS):
                g, tl = divmod(p_, NKB)
                if g == NG - 1:
                    lag = {10: 3, 11: 3, 12: 2, 13: 2, 14: 1, 15: 1}.get(tl, LAG)
                else:
                    lag = {12: 4, 13: 3, 14: 2, 15: 2}.get(tl, LAG)
                if g > 0:
                    lag = {0: 6, 1: 6, 2: 5}.get(tl, lag)
                pv_at.setdefault(p_ + lag, []).append(p_)

            for pos in range(SS + 1):
                for f in pre2.get(pos, []):
                    f()
                if pos < SS:
                    scores_step(pos)
                for p_ in pv_at.get(pos, []):
                    pv_step(p_)
                for f in pre.get(pos, []):
                    f()

            # ---- tail: group 3 normalize + qhalf-1 output blocks ----
            gl = NG - 1
            part1(gl, 0)
            warm_mms(4)
            part1(gl, 1)
            part2(gl, 0)
            warm_mms(2)
            for qb in range(8, 12):
                outproj(qb)
            part2(gl, 1)
            for qb in range(12, 16):
                outproj(qb)

    _split_multi_waits(nc)
    return nc


_NC = None


def _get_nc():
    global _NC
    if _NC is None:
        _NC = _build_nc()
    return _NC


def _prep_in_maps(Q, K, V, W_Q, W_K, W_V, W_gen_S, W_multi_head):
    bf = ml_dtypes.bfloat16
    wq_f = np.asarray(W_Q, np.float32)
    wv_f = np.asarray(W_V, np.float32)
    wm_f = np.asarray(W_multi_head, np.float32)
    # fold W_gen_S into W_K: K_hw = K @ W_K @ blockdiag(W_gen_S)
    wk_f = np.asarray(W_K, np.float32)
    wg_f = np.asarray(W_gen_S, np.float32)
    wkg_f = np.einsum('dhe,ef->dhf', wk_f.reshape(D, H, DH), wg_f)
    wkg_f = wkg_f.reshape(D, D)

    Q = np.asarray(Q, np.float32)
    K = np.asarray(K, np.float32)
    V = np.asarray(V, np.float32)

    def pack_qk(X):
        # [2048 s, 512 d] -> XT [512, 2048] -> [p, half, qc, k, 512]
        XT = X.T.reshape(ND, P, 2, 2, QC)
        return np.ascontiguousarray(XT.transpose(1, 2, 3, 0, 4)).astype(bf)

    def pack_v(X):
        # -> [p, quarter, k, 512]
        XT = X.T.reshape(ND, P, 4, QC)
        return np.ascontiguousarray(XT.transpose(1, 2, 0, 3)).astype(bf)

    def pack_w(Wl):
        # [512 d, 256 e] -> [p, k, 256]
        return np.ascontiguousarray(
            Wl.reshape(ND, P, DL).transpose(1, 0, 2)).astype(bf)

    qts = [pack_qk(Q[b]) for b in range(4)]
    kts = [pack_qk(K[b]) for b in range(4)]
    vts = [pack_v(V[b]) for b in range(4)]

    in_maps = []
    for c in range(8):
        b, hp = divmod(c, 2)
        sl = slice(hp * DL, (hp + 1) * DL)
        wm_l = wm_f[sl, :]  # [256, 512]
        in_maps.append({
            "qt": qts[b], "kt": kts[b], "vt": vts[b],
            "wq": pack_w(wq_f[:, sl]),
            "wkg": pack_w(wkg_f[:, sl]),
            "wv": pack_w(wv_f[:, sl]),
            "wm": np.ascontiguousarray(
                wm_l.reshape(NJ, P, D).transpose(1, 0, 2)).astype(bf),
        })
    return in_maps


def _run(in_maps, trace=False):
    nc = _get_nc()
    res = run_bass_kernel_spmd(nc, in_maps, list(range(8)), trace=trace)
    out = np.empty((4, SQF, D), np.float32)
    for b in range(4):
        out[b] = (res.results[2 * b]["out"].astype(np.float32)
                  + res.results[2 * b + 1]["out"].astype(np.float32))
    return out, res


def kernel(Q, K, V, M, W_Q, W_K, W_V, W_gen_S, W_multi_head):
    in_maps = _prep_in_maps(Q, K, V, W_Q, W_K, W_V, W_gen_S, W_multi_head)
    out, _ = _run(in_maps, trace=False)
    return out


def kernel_traced(Q, K, V, M, W_Q, W_K, W_V, W_gen_S, W_multi_head):
    in_maps = _prep_in_maps(Q, K, V, W_Q, W_K, W_V, W_gen_S, W_multi_head)
    return _run(in_maps, trace=True)
